# revision 4
# baseline (speedup 1.0000x reference)
"""Trainium2 Bass kernel for 16-head MHA with RoPE — zero-collective,
software-pipelined design.

Sharding: each core owns 512 output tokens (batch b = r//4, token offset
(r%4)*512 within the batch) and computes their full attention + o-proj.
The K/V projection for the core's batch (2048 keys, all 16 heads) is
replicated across the 4 cores of that batch — cheaper than any collective
through the PJRT/axon path, and removes all cross-core sync.

All per-core inputs are packed into ONE flat bf16 blob ("blob") because
per-call dispatch cost through the axon PJRT tunnel scales with the
number of argument buffers (~+0.1 ms per extra input per call measured).

The emission is software-pipelined across engines: attention for key-group
g (ScalarE-heavy: exp) is interleaved at head granularity with the K/V
projection matmuls of key-group g+1 (PE-heavy), so neither engine idles;
the o-projection is interleaved into the last group's attention as each
head-pair's output becomes available. Attention output accumulates across
key-groups in SBUF (row 64 = softmax denominator via a ones column in v).
RoPE = x*cos + (R@x)*sin with R a sign-carrying 128x128 block permutation
applied on the PE.
"""

import numpy as np
from ml_dtypes import bfloat16

# Problem shape (hardcoded per contract - kernel.py must be self-contained)
B, L_FULL, D = 2, 2048, 1024
H, HD = 16, 64
N_CORES = 8
CPB = N_CORES // B            # cores per batch = 4
KC = D // 128                 # contraction chunks = 8
OC = D // 128                 # output partition-chunks = 8


def _rope_tables(L):
    inv_freq = 1.0 / (10000.0 ** (np.arange(0, HD, 2, dtype=np.float64) / HD))
    t = np.arange(L, dtype=np.float64)
    freqs = np.outer(t, inv_freq)                      # [L, 32]
    emb = np.concatenate([freqs, freqs], -1)           # [L, 64]
    cos_t = np.cos(emb).T                              # [64, L]
    sin_t = np.sin(emb).T
    cost = np.concatenate([cos_t, cos_t], 0)           # [128, L] (2 heads)
    sint = np.concatenate([sin_t, sin_t], 0)
    return cost.astype(bfloat16), sint.astype(bfloat16)


def _rot_matrix():
    # rot(x)[o] = -x[o+32] for o in [0,32), +x[o-32] for o in [32,64),
    # per 64-row head block; stationary operand is the transpose R^T[p, o].
    rt = np.zeros((128, 128), dtype=np.float32)
    for blk in range(2):
        base = blk * 64
        for o in range(32):
            rt[base + o + 32, base + o] = -1.0
        for o in range(32, 64):
            rt[base + o - 32, base + o] = 1.0
    return rt.astype(bfloat16)


def _blob_layout(L):
    """(name -> (offset, shape)) for the packed bf16 input blob.

    Each core's key axis is rotated by its query offset (host-side
    np.roll of xt and the RoPE tables together) — softmax is invariant
    to key order, and the rotation makes the core's query tokens always
    columns [0, TPC) of xt, so no separate xq/cosq/sinq regions exist.
    """
    layout = {}
    off = 0
    for name, shape in (
        ("xt", (D, L)),
        ("wqt", (D, D)), ("wkt", (D, D)), ("wvt", (D, D)), ("wot", (D, D)),
        ("cosk", (128, L)), ("sink", (128, L)),
    ):
        layout[name] = (off, shape)
        off += int(np.prod(shape))
    return layout, off


def build_mha(tc, L=L_FULL, debug=False):
    """Emit the MHA program into TileContext `tc`.

    Per-core DRAM I/O (SPMD-uniform program; all rank differences are data):
      in : blob [NTOT] bf16 — packed xt/xq/wqt/wkt/wvt/wot/cosq/sinq
      out: y [TPC, D] f32
    """
    import concourse.mybir as mybir
    from contextlib import ExitStack

    nc = tc.nc
    f32 = mybir.dt.float32
    bf16 = mybir.dt.bfloat16
    AF = mybir.ActivationFunctionType

    T = B * L
    TPC = T // N_CORES            # query tokens per core
    MC = L // 128                 # key chunks per batch
    CH = min(512, L)              # kv-projection token chunk (key group)
    NCH = L // CH                 # key groups
    MPC = CH // 128               # key-chunks per group
    MT = min(128, TPC)            # o-proj token tile
    NMT = TPC // MT
    scale = float(HD) ** -0.5
    assert CH >= TPC or L <= 512  # queries live in group 0's columns
    # Sections redistribute the attention chunks so each section's ScalarE
    # (exp) work fits under its PE envelope: early sections carry extra kv
    # projection, the last section carries the o-projection.
    if NCH == 4:
        SEC_CHUNKS = [list(range(0, 4)), list(range(4, 12)),
                      list(range(12, 16))]
        SEC_KV = [[1, 2], [3], []]
    else:
        SEC_CHUNKS = [list(range(MC))]
        SEC_KV = [[]]

    # ---- I/O ----
    layout, ntot = _blob_layout(L)
    blob_d = nc.dram_tensor("blob", [ntot], bf16, kind="ExternalInput").ap()

    def view(name):
        off, shape = layout[name]
        v = blob_d[off:off + int(np.prod(shape))]
        return v.rearrange("(a b) -> a b", a=shape[0])

    def view_chunked(name):
        # [D, C] source seen as [128, KC, C] (partition-major chunks)
        off, shape = layout[name]
        v = blob_d[off:off + int(np.prod(shape))]
        return v.rearrange("(kk p c) -> p kk c", kk=KC, p=128)

    xt_d = view_chunked("xt")
    wqt_d = view_chunked("wqt")
    wkt_d = view_chunked("wkt")
    wvt_d = view_chunked("wvt")
    wot_d = view_chunked("wot")
    cosk_d = view("cosk")
    sink_d = view("sink")
    y_d = nc.dram_tensor("y", [TPC, D], f32, kind="ExternalOutput").ap()

    # ---- inline constants ----
    rt_d = nc.inline_tensor(_rot_matrix(), name="rotm")

    ctx = ExitStack()
    with ctx:
        # ---------------- persistent pools ----------------
        # (input DMAs are issued in consumption order: xt chunk 0 + wq feed
        # the q-projection, then cos/sin, then wk/wv/wo for the kv pipeline)
        cpool = ctx.enter_context(tc.tile_pool(name="consts", bufs=1))
        cosk = cpool.tile([128, L], bf16)
        sink = cpool.tile([128, L], bf16)
        rt_sb = cpool.tile([128, 128], bf16)

        kqpool = ctx.enter_context(tc.tile_pool(name="kq", bufs=1))
        kT = kqpool.tile([128, OC, L], bf16)    # post-RoPE k, dim-major
        qT = kqpool.tile([128, OC, TPC], bf16)  # post-RoPE q, dim-major
        vpool = ctx.enter_context(tc.tile_pool(name="vtm", bufs=1))
        v_sb = vpool.tile([128, MC, H, 65], bf16)  # v token-major + ones col
        nc.gpsimd.memset(v_sb[:, :, :, 64:65], 1.0)
        aupool = ctx.enter_context(tc.tile_pool(name="aU", bufs=1))
        aU = aupool.tile([65, H, TPC], bf16)    # attention accum across groups
        apool = ctx.enter_context(tc.tile_pool(name="aT", bufs=1))
        aT = apool.tile([128, OC, TPC], bf16)   # normalized, dim-major
        ypool = ctx.enter_context(tc.tile_pool(name="yacc", bufs=1))
        y_acc = ypool.tile([MT, NMT, D], f32)   # o-proj accum across kk

        wpool = ctx.enter_context(tc.tile_pool(name="w", bufs=1))
        wk_sb = wpool.tile([128, KC, D], bf16)
        wv_sb = wpool.tile([128, KC, D], bf16)
        wo_sb = wpool.tile([128, KC, D], bf16)

        # x stream pool (persistent; holds the current key-group chunk)
        xsp = ctx.enter_context(tc.tile_pool(name="xs", bufs=2))

        def rope_emit(ps, dst, cos_ap, sin_ap, n, rawp, up, rps):
            # dst = ps*cos + (R @ ps)*sin ; raw copy via ScalarE, rot via PE
            raw = rawp.tile([128, n], bf16, tag="raw")
            nc.vector.tensor_copy(raw[:], ps[:])
            rot = rps.tile([128, n], f32, tag="rot")
            nc.tensor.matmul(rot[:], rt_sb[:], raw[:], start=True, stop=True)
            nc.vector.tensor_mul(dst, raw[:], cos_ap)
            u = up.tile([128, n], bf16, tag="u")
            nc.vector.tensor_mul(u[:], rot[:], sin_ap)
            nc.vector.tensor_add(dst, dst, u[:])

        # ------- q-projection (queries are xt cols [0, TPC); wq freed) ------
        xt0_sb = xsp.tile([128, KC, CH], bf16, tag="xt")
        nc.sync.dma_start(xt0_sb[:, :, :], xt_d[:, :, 0:CH])
        with tc.tile_pool(name="wqp", bufs=1) as wqp, \
             tc.tile_pool(name="qraw", bufs=2) as qrawp, \
             tc.tile_pool(name="qu", bufs=2) as qup, \
             tc.tile_pool(name="qps", bufs=2, space="PSUM") as qps, \
             tc.tile_pool(name="qrps", bufs=1, space="PSUM") as qrps:
            wq_sb = wqp.tile([128, KC, D], bf16)
            nc.sync.dma_start(wq_sb[:, :, :], wqt_d[:, :, :])
            nc.sync.dma_start(cosk[:], cosk_d[:, :])
            nc.sync.dma_start(sink[:], sink_d[:, :])
            nc.sync.dma_start(rt_sb[:], rt_d.ap()[:, :])
            nc.sync.dma_start(wk_sb[:, :, :], wkt_d[:, :, :])
            nc.sync.dma_start(wv_sb[:, :, :], wvt_d[:, :, :])
            nc.sync.dma_start(wo_sb[:, :, :], wot_d[:, :, :])
            for oc in range(OC):
                q_ps = qps.tile([128, TPC], f32, tag="q_ps")
                for kk in range(KC):
                    nc.tensor.matmul(q_ps[:],
                                     wq_sb[:, kk, oc * 128:(oc + 1) * 128],
                                     xt0_sb[:, kk, 0:TPC],
                                     start=(kk == 0), stop=(kk == KC - 1))
                rope_emit(q_ps, qT[:, oc, :], cosk[:, 0:TPC], sink[:, 0:TPC],
                          TPC, qrawp, qup, qrps)

        # attention pools (outlive the kv pools; closed after the last group)
        attn_stack = ExitStack()
        ptp = attn_stack.enter_context(tc.tile_pool(name="pt", bufs=3))
        epool = attn_stack.enter_context(tc.tile_pool(name="ep", bufs=2))
        stp = attn_stack.enter_context(tc.tile_pool(name="stp", bufs=2, space="PSUM"))
        oup = attn_stack.enter_context(tc.tile_pool(name="oup", bufs=1, space="PSUM"))

        # kv-projection pools (closed once the last group is projected)
        kvstack = ExitStack()
        rawp = kvstack.enter_context(tc.tile_pool(name="raw", bufs=2))
        up = kvstack.enter_context(tc.tile_pool(name="u", bufs=2))
        kps = kvstack.enter_context(tc.tile_pool(name="kps", bufs=2, space="PSUM"))
        rps = kvstack.enter_context(tc.tile_pool(name="rps", bufs=1, space="PSUM"))
        vps = kvstack.enter_context(tc.tile_pool(name="vps", bufs=1, space="PSUM"))

        def emit_kv_pieces(g, xt_pre=None):
            """Generator of closures; each emits one PE-sized piece of the
            K/V projection + RoPE for key group g."""
            cols = slice(g * CH, (g + 1) * CH)
            if xt_pre is None:
                xt_sb = xsp.tile([128, KC, CH], bf16, tag="xt")

                def load():
                    nc.sync.dma_start(xt_sb[:, :, :], xt_d[:, :, cols])
                yield load
            else:
                xt_sb = xt_pre
            for oc in range(OC):
                def kproj(oc=oc):
                    k_ps = kps.tile([128, CH], f32, tag="qk_ps")
                    for kk in range(KC):
                        nc.tensor.matmul(k_ps[:],
                                         wk_sb[:, kk, oc * 128:(oc + 1) * 128],
                                         xt_sb[:, kk, :],
                                         start=(kk == 0), stop=(kk == KC - 1))
                    rope_emit(k_ps, kT[:, oc, cols], cosk[:, cols],
                              sink[:, cols], CH, rawp, up, rps)
                yield kproj
            for mi in range(MPC):
                def vproj(mi=mi):
                    m = g * MPC + mi
                    ts = slice(mi * 128, (mi + 1) * 128)
                    v_ps = vps.tile([128, H, 64], f32, tag="v_ps")
                    for kk in range(KC):
                        for hf in range(2):
                            nc.tensor.matmul(
                                v_ps[:, hf * 8:(hf + 1) * 8, :],
                                xt_sb[:, kk, ts],
                                wv_sb[:, kk, hf * 512:(hf + 1) * 512],
                                start=(kk == 0), stop=(kk == KC - 1))
                    nc.vector.tensor_copy(v_sb[:, m, :, 0:64], v_ps[:])
                yield vproj

        # ---------------- pipelined attention + kv + o-proj ----------------
        def attn_head(chunks, first, h):
            """Scores + exp + PV for head h over the listed key chunks;
            accumulate into aU[:, h, :]."""
            po, pc = (h % 2) * 64, h // 2
            hs = slice(po, po + 64)
            outU = oup.tile([65, TPC], f32, tag="outU")
            pend = None
            for j, m in enumerate(chunks):
                ks = slice(m * 128, (m + 1) * 128)
                st = stp.tile([128, TPC], f32, tag="st")
                nc.tensor.matmul(st[:], kT[hs, pc, ks], qT[hs, pc, :],
                                 start=True, stop=True)
                pt = ptp.tile([128, TPC], bf16, tag="pt")
                nc.scalar.activation(pt[:], st[:], AF.Exp, scale=scale)
                if pend is not None:
                    pj, pm, ppt = pend
                    nc.tensor.matmul(outU[:], v_sb[:, pm, h, :], ppt[:],
                                     start=(pj == 0), stop=False)
                pend = (j, m, pt)
            pj, pm, ppt = pend
            nc.tensor.matmul(outU[:], v_sb[:, pm, h, :], ppt[:],
                             start=(pj == 0), stop=True)
            if first:
                nc.vector.tensor_copy(aU[:, h, :], outU[:])
            else:
                # bf16 accumulation over only 3 partial sums; error is
                # ~0.4% against a 2e-2 correctness budget
                with nc.allow_low_precision(reason="3-way bf16 attn accum"):
                    nc.vector.tensor_add(aU[:, h, :], aU[:, h, :], outU[:])

        def normalize_quad(q):
            # batched reciprocal of 4 heads' denominators into a partition-0
            # tile (partition_broadcast sources partition 0 on hardware),
            # then broadcast + scale each head into dim-major aT (all bf16)
            hq = slice(4 * q, 4 * q + 4)
            dinv = epool.tile([1, 4, TPC], bf16, tag="dinv")
            with nc.allow_low_precision(reason="bf16 softmax denom recip"):
                nc.vector.reciprocal(dinv[:], aU[64:65, hq, :])
            for j, h in enumerate(range(4 * q, 4 * q + 4)):
                po, pc = (h % 2) * 64, h // 2
                hs = slice(po, po + 64)
                bc = epool.tile([64, TPC], bf16, tag="bc")
                nc.gpsimd.partition_broadcast(bc[:], dinv[0:1, j, :])
                nc.vector.tensor_mul(aT[hs, pc, :], aU[0:64, h, :], bc[:])

        # group 0 kv-projection runs un-overlapped (nothing to hide it behind)
        for piece in emit_kv_pieces(0, xt_pre=xt0_sb):
            piece()

        yps_stack = ExitStack()

        def oproj_piece(half, yps):
            """o-projection over a kk-quad (heads 8*half..8*half+7), PSUM
            accumulated, then one add per token tile into y_acc."""
            kks = range(4 * half, 4 * half + 4)
            for mt in range(NMT):
                ms = slice(mt * MT, (mt + 1) * MT)
                y_ps = yps.tile([MT, D], f32, tag="y_ps")
                for j, kk in enumerate(kks):
                    for no in range(2):
                        nc.tensor.matmul(y_ps[:, no * 512:(no + 1) * 512],
                                         aT[:, kk, ms],
                                         wo_sb[:, kk, no * 512:(no + 1) * 512],
                                         start=(j == 0), stop=(j == 3))
                if half == 0:
                    nc.vector.tensor_copy(y_acc[:, mt, :], y_ps[:])
                else:
                    nc.vector.tensor_add(y_acc[:, mt, :], y_acc[:, mt, :],
                                         y_ps[:])

        for s, chunks in enumerate(SEC_CHUNKS):
            last = (s == len(SEC_CHUNKS) - 1)
            pieces = []
            for g in SEC_KV[s]:
                pieces.extend(emit_kv_pieces(g))
            if last:
                # all projections emitted; free kv PSUM for o-proj tiles
                kvstack.close()
                yps = yps_stack.enter_context(
                    tc.tile_pool(name="yps", bufs=2, space="PSUM"))
            pi = 0
            npc = len(pieces)
            for h in range(H):
                attn_head(chunks, s == 0, h)
                # spread the kv pieces of later groups evenly over the heads
                want = (h + 1) * npc // H
                while pi < want:
                    pieces[pi]()
                    pi += 1
                if last:
                    if h % 4 == 3:
                        normalize_quad(h // 4)
                    if h % 8 == 7:
                        oproj_piece(h // 8, yps)

        yps_stack.close()
        attn_stack.close()

        # ---------------- y writeback ----------------
        for mt in range(NMT):
            nc.sync.dma_start(y_d[mt * MT:(mt + 1) * MT, :], y_acc[:, mt, :])

    return nc


def make_in_maps(x, wq, wk, wv, wo, L=L_FULL):
    T = B * L
    TPC = T // N_CORES
    x3 = np.asarray(x, dtype=np.float32).reshape(B, L, D)
    xt_b = [np.ascontiguousarray(x3[b].T).astype(bfloat16) for b in range(B)]
    wqt = np.ascontiguousarray(np.asarray(wq, np.float32).T).astype(bfloat16)
    wkt = np.ascontiguousarray(np.asarray(wk, np.float32).T).astype(bfloat16)
    wvt = np.ascontiguousarray(np.asarray(wv, np.float32).T).astype(bfloat16)
    wot = np.ascontiguousarray(np.asarray(wo, np.float32).T).astype(bfloat16)
    cost_np, sint_np = _rope_tables(L)
    layout, ntot = _blob_layout(L)
    in_maps = []
    for r in range(N_CORES):
        b = r // CPB
        qoff = (r % CPB) * TPC
        # rotate the key axis so this core's queries sit at columns [0, TPC);
        # softmax is key-order invariant, so rotating x and the RoPE tables
        # together is exact.
        parts = {
            "xt": np.roll(xt_b[b], -qoff, axis=1),
            "wqt": wqt, "wkt": wkt, "wvt": wvt, "wot": wot,
            "cosk": np.roll(cost_np, -qoff, axis=1),
            "sink": np.roll(sint_np, -qoff, axis=1),
        }
        blob = np.empty(ntot, dtype=bfloat16)
        for name, (off, shape) in layout.items():
            blob[off:off + int(np.prod(shape))] = np.ascontiguousarray(
                parts[name]).ravel()
        in_maps.append({"blob": blob})
    return in_maps


_BUILT = {}


def _get_nc(L=L_FULL):
    if L not in _BUILT:
        import concourse.tile as tile
        from concourse import bacc
        nc = bacc.Bacc(num_devices=N_CORES)
        with tile.TileContext(nc) as tc:
            build_mha(tc, L=L)
        nc.compile()
        _BUILT[L] = nc
    return _BUILT[L]


def kernel(x, wq, wk, wv, wo):
    from concourse.bass_utils import run_bass_kernel_spmd
    nc = _get_nc()
    in_maps = make_in_maps(x, wq, wk, wv, wo)
    res = run_bass_kernel_spmd(nc, in_maps, core_ids=list(range(N_CORES)))
    TPC = B * L_FULL // N_CORES
    y = np.empty((B, L_FULL, D), np.float32)
    for r in range(N_CORES):
        b = r // CPB
        qoff = (r % CPB) * TPC
        y[b, qoff:qoff + TPC] = res.results[r]["y"]
    return y


# revision 5
# speedup vs baseline: 1.1263x; 1.1263x over previous
"""Trainium2 Bass kernel for 16-head MHA with RoPE — zero-collective,
software-pipelined design.

Sharding: each core owns 512 output tokens (batch b = r//4, token offset
(r%4)*512 within the batch) and computes their full attention + o-proj.
The K/V projection for the core's batch (2048 keys, all 16 heads) is
replicated across the 4 cores of that batch — cheaper than any collective
through the PJRT/axon path, and removes all cross-core sync.

All per-core inputs are packed into ONE flat bf16 blob ("blob") because
per-call dispatch cost through the axon PJRT tunnel scales with the
number of argument buffers (~+0.1 ms per extra input per call measured).

The emission is software-pipelined across engines: attention for key-group
g (ScalarE-heavy: exp) is interleaved at head granularity with the K/V
projection matmuls of key-group g+1 (PE-heavy), so neither engine idles;
the o-projection is interleaved into the last group's attention as each
head-pair's output becomes available. Attention output accumulates across
key-groups in SBUF (row 64 = softmax denominator via a ones column in v).
RoPE = x*cos + (R@x)*sin with R a sign-carrying 128x128 block permutation
applied on the PE.
"""

import numpy as np
from ml_dtypes import bfloat16

# Problem shape (hardcoded per contract - kernel.py must be self-contained)
B, L_FULL, D = 2, 2048, 1024
H, HD = 16, 64
N_CORES = 8
CPB = N_CORES // B            # cores per batch = 4
KC = D // 128                 # contraction chunks = 8
OC = D // 128                 # output partition-chunks = 8


def _rope_tables(L):
    inv_freq = 1.0 / (10000.0 ** (np.arange(0, HD, 2, dtype=np.float64) / HD))
    t = np.arange(L, dtype=np.float64)
    freqs = np.outer(t, inv_freq)                      # [L, 32]
    emb = np.concatenate([freqs, freqs], -1)           # [L, 64]
    cos_t = np.cos(emb).T                              # [64, L]
    sin_t = np.sin(emb).T
    cost = np.concatenate([cos_t, cos_t], 0)           # [128, L] (2 heads)
    sint = np.concatenate([sin_t, sin_t], 0)
    return cost.astype(bfloat16), sint.astype(bfloat16)


def _rot_matrix():
    # rot(x)[o] = -x[o+32] for o in [0,32), +x[o-32] for o in [32,64),
    # per 64-row head block; stationary operand is the transpose R^T[p, o].
    rt = np.zeros((128, 128), dtype=np.float32)
    for blk in range(2):
        base = blk * 64
        for o in range(32):
            rt[base + o + 32, base + o] = -1.0
        for o in range(32, 64):
            rt[base + o - 32, base + o] = 1.0
    return rt.astype(bfloat16)


def _blob_layout(L):
    """(name -> (offset, shape)) for the packed bf16 input blob.

    Each core's key axis is rotated by its query offset (host-side
    np.roll of xt and the RoPE tables together) — softmax is invariant
    to key order, and the rotation makes the core's query tokens always
    columns [0, TPC) of xt, so no separate xq/cosq/sinq regions exist.
    """
    layout = {}
    off = 0
    for name, shape in (
        ("xt", (D, L)),
        ("wqt", (D, D)), ("wkt", (D, D)), ("wvt", (D, D)), ("wot", (D, D)),
        ("cosk", (128, L)), ("sink", (128, L)),
    ):
        layout[name] = (off, shape)
        off += int(np.prod(shape))
    return layout, off


def build_mha(tc, L=L_FULL, debug=False):
    """Emit the MHA program into TileContext `tc`.

    Per-core DRAM I/O (SPMD-uniform program; all rank differences are data):
      in : blob [NTOT] bf16 — packed xt/xq/wqt/wkt/wvt/wot/cosq/sinq
      out: y [TPC, D] f32
    """
    import concourse.mybir as mybir
    from contextlib import ExitStack

    nc = tc.nc
    f32 = mybir.dt.float32
    bf16 = mybir.dt.bfloat16
    AF = mybir.ActivationFunctionType

    T = B * L
    TPC = T // N_CORES            # query tokens per core
    MC = L // 128                 # key chunks per batch
    CH = min(512, L)              # kv-projection token chunk (key group)
    NCH = L // CH                 # key groups
    MPC = CH // 128               # key-chunks per group
    MT = min(128, TPC)            # o-proj token tile
    NMT = TPC // MT
    scale = float(HD) ** -0.5
    assert CH >= TPC or L <= 512  # queries live in group 0's columns
    # Sections redistribute the attention chunks so each section's ScalarE
    # (exp) work fits under its PE envelope: early sections carry extra kv
    # projection, the last section carries the o-projection.
    if NCH == 4:
        SEC_CHUNKS = [list(range(0, 4)), list(range(4, 12)),
                      list(range(12, 16))]
        SEC_KV = [[1, 2], [3], []]
    else:
        SEC_CHUNKS = [list(range(MC))]
        SEC_KV = [[]]

    # ---- I/O ----
    layout, ntot = _blob_layout(L)
    blob_d = nc.dram_tensor("blob", [ntot], bf16, kind="ExternalInput").ap()

    def view(name):
        off, shape = layout[name]
        v = blob_d[off:off + int(np.prod(shape))]
        return v.rearrange("(a b) -> a b", a=shape[0])

    def view_chunked(name):
        # [D, C] source seen as [128, KC, C] (partition-major chunks)
        off, shape = layout[name]
        v = blob_d[off:off + int(np.prod(shape))]
        return v.rearrange("(kk p c) -> p kk c", kk=KC, p=128)

    xt_d = view_chunked("xt")
    wqt_d = view_chunked("wqt")
    wkt_d = view_chunked("wkt")
    wvt_d = view_chunked("wvt")
    wot_d = view_chunked("wot")
    cosk_d = view("cosk")
    sink_d = view("sink")
    y_d = nc.dram_tensor("y", [TPC, D], f32, kind="ExternalOutput").ap()

    # ---- inline constants ----
    rt_d = nc.inline_tensor(_rot_matrix(), name="rotm")

    ctx = ExitStack()
    with ctx:
        # ---------------- persistent pools ----------------
        # (input DMAs are issued in consumption order: xt chunk 0 + wq feed
        # the q-projection, then cos/sin, then wk/wv/wo for the kv pipeline)
        cpool = ctx.enter_context(tc.tile_pool(name="consts", bufs=1))
        cosk = cpool.tile([128, L], bf16)
        sink = cpool.tile([128, L], bf16)
        rt_sb = cpool.tile([128, 128], bf16)

        kqpool = ctx.enter_context(tc.tile_pool(name="kq", bufs=1))
        kT = kqpool.tile([128, OC, L], bf16)    # post-RoPE k, dim-major
        qT = kqpool.tile([128, OC, TPC], bf16)  # post-RoPE q, dim-major
        vpool = ctx.enter_context(tc.tile_pool(name="vtm", bufs=1))
        v_sb = vpool.tile([128, MC, H, 65], bf16)  # v token-major + ones col
        nc.gpsimd.memset(v_sb[:, :, :, 64:65], 1.0)
        aupool = ctx.enter_context(tc.tile_pool(name="aU", bufs=1))
        aU = aupool.tile([65, H, TPC], bf16)    # attention accum across groups
        apool = ctx.enter_context(tc.tile_pool(name="aT", bufs=1))
        aT = apool.tile([128, OC, TPC], bf16)   # normalized, dim-major
        ypool = ctx.enter_context(tc.tile_pool(name="yacc", bufs=1))
        y_acc = ypool.tile([MT, NMT, D], f32)   # o-proj accum across kk

        wpool = ctx.enter_context(tc.tile_pool(name="w", bufs=1))
        wk_sb = wpool.tile([128, KC, D], bf16)
        wv_sb = wpool.tile([128, KC, D], bf16)
        wo_sb = wpool.tile([128, KC, D], bf16)

        # x stream pool (persistent; holds the current key-group chunk)
        xsp = ctx.enter_context(tc.tile_pool(name="xs", bufs=2))

        def rope_emit(ps, dst, cos_ap, sin_ap, n, rawp, up, rps):
            # dst = ps*cos + (R @ ps)*sin ; raw copy via ScalarE, rot via PE
            raw = rawp.tile([128, n], bf16, tag="raw")
            nc.vector.tensor_copy(raw[:], ps[:])
            rot = rps.tile([128, n], f32, tag="rot")
            nc.tensor.matmul(rot[:], rt_sb[:], raw[:], start=True, stop=True)
            nc.vector.tensor_mul(dst, raw[:], cos_ap)
            u = up.tile([128, n], bf16, tag="u")
            nc.vector.tensor_mul(u[:], rot[:], sin_ap)
            nc.vector.tensor_add(dst, dst, u[:])

        # ------- q-projection (queries are xt cols [0, TPC); wq freed) ------
        xt0_sb = xsp.tile([128, KC, CH], bf16, tag="xt")
        with tc.tile_pool(name="wqp", bufs=1) as wqp, \
             tc.tile_pool(name="qraw", bufs=2) as qrawp, \
             tc.tile_pool(name="qu", bufs=2) as qup, \
             tc.tile_pool(name="qps", bufs=2, space="PSUM") as qps, \
             tc.tile_pool(name="qrps", bufs=1, space="PSUM") as qrps:
            wq_sb = wqp.tile([128, KC, D], bf16)
            # per-chunk loads, interleaved: the first q-proj matmul only
            # needs kk=0 of xt0 and wq (~400 KB), not the full 4 MB
            for kk in range(KC):
                nc.sync.dma_start(xt0_sb[:, kk, :], xt_d[:, kk, 0:CH])
                nc.sync.dma_start(wq_sb[:, kk, :], wqt_d[:, kk, :])
            nc.sync.dma_start(cosk[:], cosk_d[:, :])
            nc.sync.dma_start(sink[:], sink_d[:, :])
            nc.sync.dma_start(rt_sb[:], rt_d.ap()[:, :])
            nc.sync.dma_start(wk_sb[:, :, :], wkt_d[:, :, :])
            nc.sync.dma_start(wv_sb[:, :, :], wvt_d[:, :, :])
            nc.sync.dma_start(wo_sb[:, :, :], wot_d[:, :, :])
            for oc in range(OC):
                q_ps = qps.tile([128, TPC], f32, tag="q_ps")
                for kk in range(KC):
                    nc.tensor.matmul(q_ps[:],
                                     wq_sb[:, kk, oc * 128:(oc + 1) * 128],
                                     xt0_sb[:, kk, 0:TPC],
                                     start=(kk == 0), stop=(kk == KC - 1))
                rope_emit(q_ps, qT[:, oc, :], cosk[:, 0:TPC], sink[:, 0:TPC],
                          TPC, qrawp, qup, qrps)

        # attention pools (outlive the kv pools; closed after the last group)
        attn_stack = ExitStack()
        ptp = attn_stack.enter_context(tc.tile_pool(name="pt", bufs=3))
        epool = attn_stack.enter_context(tc.tile_pool(name="ep", bufs=2))
        stp = attn_stack.enter_context(tc.tile_pool(name="stp", bufs=2, space="PSUM"))
        oup = attn_stack.enter_context(tc.tile_pool(name="oup", bufs=1, space="PSUM"))

        # kv-projection pools (closed once the last group is projected)
        kvstack = ExitStack()
        rawp = kvstack.enter_context(tc.tile_pool(name="raw", bufs=2))
        up = kvstack.enter_context(tc.tile_pool(name="u", bufs=2))
        kps = kvstack.enter_context(tc.tile_pool(name="kps", bufs=2, space="PSUM"))
        rps = kvstack.enter_context(tc.tile_pool(name="rps", bufs=1, space="PSUM"))
        vps = kvstack.enter_context(tc.tile_pool(name="vps", bufs=1, space="PSUM"))

        def emit_kv_pieces(g, xt_pre=None):
            """Generator of closures; each emits one PE-sized piece of the
            K/V projection + RoPE for key group g."""
            cols = slice(g * CH, (g + 1) * CH)
            if xt_pre is None:
                xt_sb = xsp.tile([128, KC, CH], bf16, tag="xt")

                def load():
                    nc.sync.dma_start(xt_sb[:, :, :], xt_d[:, :, cols])
                yield load
            else:
                xt_sb = xt_pre
            for oc in range(OC):
                def kproj(oc=oc):
                    k_ps = kps.tile([128, CH], f32, tag="qk_ps")
                    for kk in range(KC):
                        nc.tensor.matmul(k_ps[:],
                                         wk_sb[:, kk, oc * 128:(oc + 1) * 128],
                                         xt_sb[:, kk, :],
                                         start=(kk == 0), stop=(kk == KC - 1))
                    rope_emit(k_ps, kT[:, oc, cols], cosk[:, cols],
                              sink[:, cols], CH, rawp, up, rps)
                yield kproj
            for mi in range(MPC):
                def vproj(mi=mi):
                    m = g * MPC + mi
                    ts = slice(mi * 128, (mi + 1) * 128)
                    v_ps = vps.tile([128, H, 64], f32, tag="v_ps")
                    for kk in range(KC):
                        for hf in range(2):
                            nc.tensor.matmul(
                                v_ps[:, hf * 8:(hf + 1) * 8, :],
                                xt_sb[:, kk, ts],
                                wv_sb[:, kk, hf * 512:(hf + 1) * 512],
                                start=(kk == 0), stop=(kk == KC - 1))
                    nc.vector.tensor_copy(v_sb[:, m, :, 0:64], v_ps[:])
                yield vproj

        # ---------------- pipelined attention + kv + o-proj ----------------
        def attn_head(chunks, first, h):
            """Scores + exp + PV for head h over the listed key chunks;
            accumulate into aU[:, h, :]."""
            po, pc = (h % 2) * 64, h // 2
            hs = slice(po, po + 64)
            outU = oup.tile([65, TPC], f32, tag="outU")
            pend = None
            for j, m in enumerate(chunks):
                ks = slice(m * 128, (m + 1) * 128)
                st = stp.tile([128, TPC], f32, tag="st")
                nc.tensor.matmul(st[:], kT[hs, pc, ks], qT[hs, pc, :],
                                 start=True, stop=True)
                pt = ptp.tile([128, TPC], bf16, tag="pt")
                nc.scalar.activation(pt[:], st[:], AF.Exp, scale=scale)
                if pend is not None:
                    pj, pm, ppt = pend
                    nc.tensor.matmul(outU[:], v_sb[:, pm, h, :], ppt[:],
                                     start=(pj == 0), stop=False)
                pend = (j, m, pt)
            pj, pm, ppt = pend
            nc.tensor.matmul(outU[:], v_sb[:, pm, h, :], ppt[:],
                             start=(pj == 0), stop=True)
            if first:
                nc.vector.tensor_copy(aU[:, h, :], outU[:])
            else:
                # bf16 accumulation over only 3 partial sums; error is
                # ~0.4% against a 2e-2 correctness budget
                with nc.allow_low_precision(reason="3-way bf16 attn accum"):
                    nc.vector.tensor_add(aU[:, h, :], aU[:, h, :], outU[:])

        def normalize_quad(q):
            # batched reciprocal of 4 heads' denominators into a partition-0
            # tile (partition_broadcast sources partition 0 on hardware),
            # then broadcast + scale each head into dim-major aT (all bf16)
            hq = slice(4 * q, 4 * q + 4)
            dinv = epool.tile([1, 4, TPC], bf16, tag="dinv")
            with nc.allow_low_precision(reason="bf16 softmax denom recip"):
                nc.vector.reciprocal(dinv[:], aU[64:65, hq, :])
            for j, h in enumerate(range(4 * q, 4 * q + 4)):
                po, pc = (h % 2) * 64, h // 2
                hs = slice(po, po + 64)
                bc = epool.tile([64, TPC], bf16, tag="bc")
                nc.gpsimd.partition_broadcast(bc[:], dinv[0:1, j, :])
                nc.vector.tensor_mul(aT[hs, pc, :], aU[0:64, h, :], bc[:])

        # group 0 kv-projection runs un-overlapped (nothing to hide it behind)
        for piece in emit_kv_pieces(0, xt_pre=xt0_sb):
            piece()

        yps_stack = ExitStack()

        def oproj_piece(half, yps):
            """o-projection over a kk-quad (heads 8*half..8*half+7), PSUM
            accumulated, then one add per token tile into y_acc."""
            kks = range(4 * half, 4 * half + 4)
            for mt in range(NMT):
                ms = slice(mt * MT, (mt + 1) * MT)
                y_ps = yps.tile([MT, D], f32, tag="y_ps")
                for j, kk in enumerate(kks):
                    for no in range(2):
                        nc.tensor.matmul(y_ps[:, no * 512:(no + 1) * 512],
                                         aT[:, kk, ms],
                                         wo_sb[:, kk, no * 512:(no + 1) * 512],
                                         start=(j == 0), stop=(j == 3))
                if half == 0:
                    nc.vector.tensor_copy(y_acc[:, mt, :], y_ps[:])
                else:
                    nc.vector.tensor_add(y_acc[:, mt, :], y_acc[:, mt, :],
                                         y_ps[:])

        for s, chunks in enumerate(SEC_CHUNKS):
            last = (s == len(SEC_CHUNKS) - 1)
            pieces = []
            for g in SEC_KV[s]:
                pieces.extend(emit_kv_pieces(g))
            if last:
                # all projections emitted; free kv PSUM for o-proj tiles
                kvstack.close()
                yps = yps_stack.enter_context(
                    tc.tile_pool(name="yps", bufs=2, space="PSUM"))
            pi = 0
            npc = len(pieces)
            for h in range(H):
                attn_head(chunks, s == 0, h)
                # spread the kv pieces of later groups evenly over the heads
                want = (h + 1) * npc // H
                while pi < want:
                    pieces[pi]()
                    pi += 1
                if last:
                    if h % 4 == 3:
                        normalize_quad(h // 4)
                    if h % 8 == 7:
                        oproj_piece(h // 8, yps)

        yps_stack.close()
        attn_stack.close()

        # ---------------- y writeback ----------------
        for mt in range(NMT):
            nc.sync.dma_start(y_d[mt * MT:(mt + 1) * MT, :], y_acc[:, mt, :])

    return nc


def make_in_maps(x, wq, wk, wv, wo, L=L_FULL):
    T = B * L
    TPC = T // N_CORES
    x3 = np.asarray(x, dtype=np.float32).reshape(B, L, D)
    xt_b = [np.ascontiguousarray(x3[b].T).astype(bfloat16) for b in range(B)]
    wqt = np.ascontiguousarray(np.asarray(wq, np.float32).T).astype(bfloat16)
    wkt = np.ascontiguousarray(np.asarray(wk, np.float32).T).astype(bfloat16)
    wvt = np.ascontiguousarray(np.asarray(wv, np.float32).T).astype(bfloat16)
    wot = np.ascontiguousarray(np.asarray(wo, np.float32).T).astype(bfloat16)
    cost_np, sint_np = _rope_tables(L)
    layout, ntot = _blob_layout(L)
    in_maps = []
    for r in range(N_CORES):
        b = r // CPB
        qoff = (r % CPB) * TPC
        # rotate the key axis so this core's queries sit at columns [0, TPC);
        # softmax is key-order invariant, so rotating x and the RoPE tables
        # together is exact.
        parts = {
            "xt": np.roll(xt_b[b], -qoff, axis=1),
            "wqt": wqt, "wkt": wkt, "wvt": wvt, "wot": wot,
            "cosk": np.roll(cost_np, -qoff, axis=1),
            "sink": np.roll(sint_np, -qoff, axis=1),
        }
        blob = np.empty(ntot, dtype=bfloat16)
        for name, (off, shape) in layout.items():
            blob[off:off + int(np.prod(shape))] = np.ascontiguousarray(
                parts[name]).ravel()
        in_maps.append({"blob": blob})
    return in_maps


_BUILT = {}


def _get_nc(L=L_FULL):
    if L not in _BUILT:
        import concourse.tile as tile
        from concourse import bacc
        nc = bacc.Bacc(num_devices=N_CORES)
        with tile.TileContext(nc) as tc:
            build_mha(tc, L=L)
        nc.compile()
        _BUILT[L] = nc
    return _BUILT[L]


def kernel(x, wq, wk, wv, wo):
    from concourse.bass_utils import run_bass_kernel_spmd
    nc = _get_nc()
    in_maps = make_in_maps(x, wq, wk, wv, wo)
    res = run_bass_kernel_spmd(nc, in_maps, core_ids=list(range(N_CORES)))
    TPC = B * L_FULL // N_CORES
    y = np.empty((B, L_FULL, D), np.float32)
    for r in range(N_CORES):
        b = r // CPB
        qoff = (r % CPB) * TPC
        y[b, qoff:qoff + TPC] = res.results[r]["y"]
    return y
